# revision 42
# baseline (speedup 1.0000x reference)
"""Trainium2 Bass kernel for nn_ExpansionContrastModule.

Strategy (pure data-parallel, one batch per NeuronCore, 8 cores):
  - Fuse the depthwise contrast kernels + 1x1 k_w/v_w projections into
    dense per-shift 3x3 conv weights on the host (tiny). Image tiles are
    bf16; each conv is 5 K=128 matmuls per 4-row chunk: 3 "V-pair"
    matmuls (partitions 64-127 hold the padded image shifted down by
    2*d rows -> taps ky=-d/+d), 1 "H-pair" matmul (upper half shifted
    left by 2*d cols -> taps kx=-d/+d at ky=0), 1 solo (center tap).
  - K chunks are evicted as bf16, PE-transposed, and immediately folded
    into the attention score Gram matrix (contraction over all 9216
    spatial positions); K is never materialized in full.
  - L2 normalization of Q/K is folded into the tiny score matrix via
    row/col norm scalars. InstanceNorm + softmax on-chip.
  - V phase: out_w is folded into attention (C_i = out_w @ attn_i) and,
    for shifts d=1,2,4, C_i is further folded into the V conv weights on
    device (DT_im = WvT_im @ C_i^T), so those convs write y directly
    from 5 matmuls per chunk. The d=8 shift runs the unfused path first,
    keeping the PE busy while attention/C/DT compute.
  - y lives as [128, 4608] (chunk-interleaved halves) so accumulation,
    stats and ReLU use all 128 partitions.
  - Train-mode BatchNorm stats are AllReduced across the 8 cores.
Heads live at 32-aligned partition bases (rows 32g+q, q<16) so the
per-head stat matmuls satisfy base-partition rules.
"""
import os
os.environ.setdefault("JAX_PLATFORMS", "axon,cpu")
import numpy as np

SHIFTS = (1, 2, 4, 8)
NL = 8
B, C, H, W = 8, 64, 96, 96
NH, HID = 4, 16
S = H * W
PW, PAD = 112, 8
TROWS = 104             # padded rows kept on device (reads stay below 104)
NT = 384                 # conv chunk: 4 rows of 96
NCH = S // NT            # 24 chunks
NG = NCH // 2            # 12 paired chunk groups in the y layout
NMM = 5                  # matmuls per conv chunk (3 V-pairs, 1 H-pair, 1 solo)
NCORES = 8
EPS = 1e-5

# ---------------------------------------------------------------- host math
def _softmax(x, axis):
    m = x.max(axis=axis, keepdims=True)
    e = np.exp(x - m)
    return e / e.sum(axis=axis, keepdims=True)


def _host_weights(sum_weights, q_w, k_w, v_w, out_w):
    import ml_dtypes
    d1 = np.array([[[-1, 0, 0], [0, 1, 0], [0, 0, 0]],
                   [[0, -1, 0], [0, 1, 0], [0, 0, 0]],
                   [[0, 0, -1], [0, 1, 0], [0, 0, 0]],
                   [[0, 0, 0], [0, 1, -1], [0, 0, 0]]], dtype=np.float32).reshape(4, 1, 3, 3)
    d2 = d1[:, :, ::-1, ::-1].copy()
    K8 = np.concatenate([d1, d2], 0)[:, 0].astype(np.float64)   # [8,3,3]
    K0 = K8.mean(0)
    wconv = np.zeros((128, 2, NH, NMM, 128), np.float64)        # [part, kv, shift, mm, o]
    for i in range(NH):
        sw = _softmax(sum_weights[i].astype(np.float64), -1).reshape(C, 2)
        mixed = (K8[:, None] * sw[None, :, 0, None, None]
                 + K0[None, None] * sw[None, :, 1, None, None])  # [8,C,3,3]
        for kv, w in ((0, k_w), (1, v_w)):
            Weff = np.einsum('ojc,jcyx->ocyx',
                             w[i].astype(np.float64).reshape(128, NL, C), mixed)
            for m in range(3):      # V-pairs: kx index m, ky=0 lower / ky=2 upper
                wconv[0:64, kv, i, m, :] = Weff[:, :, 0, m].T
                wconv[64:128, kv, i, m, :] = Weff[:, :, 2, m].T
            # H-pair: ky=1 row, kx=-d lower / kx=+d upper
            wconv[0:64, kv, i, 3, :] = Weff[:, :, 1, 0].T
            wconv[64:128, kv, i, 3, :] = Weff[:, :, 1, 2].T
            # solo: center tap, upper zero
            wconv[0:64, kv, i, 4, :] = Weff[:, :, 1, 1].T
    bf16 = ml_dtypes.bfloat16
    # transposed V-conv weight blocks for the on-device D-fold (shifts 0..2)
    wcvT = np.zeros((128, 3, NMM, 128), np.float64)
    for i in range(3):
        for m in range(NMM):
            wcvT[:, i, m, :] = wconv[:, 1, i, m, :].T
    wq = np.zeros((128, 128), np.float64)        # lhsT [c2, q2]; rows>=64 zero
    for f in range(64):
        g, q = divmod(f, 16)
        wq[0:C, 32 * g + q] = q_w[f % 4, f // 4]
    outwp = np.zeros((128, 64), np.float32)      # lhsT rows c2=32g+q -> out_w[:, g*16+q]
    for c in range(64):
        g, q = divmod(c, 16)
        outwp[32 * g + q, :] = out_w[:, c]
    stack2 = np.zeros((64, 128), np.float32)     # lhsT: broadcast [64] -> [128] halves
    for c in range(64):
        stack2[c, c] = 1.0
        stack2[c, 64 + c] = 1.0
    ind4 = np.zeros((128, 4), np.float32)        # lhsT: per-head partition reduce
    ind4T = np.zeros((4, 128), np.float32)       # lhsT: per-head partition broadcast
    for g in range(4):
        ind4[32 * g:32 * g + 16, g] = 1.0
        ind4T[g, 32 * g:32 * g + 16] = 1.0
    return (wconv[:, 0].reshape(128, NH * NMM * 128).astype(bf16),
            wconv[:, 1].reshape(128, NH * NMM * 128).astype(bf16),
            wcvT.reshape(128, 3 * NMM * 128).astype(bf16),
            wq.astype(bf16), outwp, stack2, ind4, ind4T)


# ---------------------------------------------------------------- device build
_CACHE = {}


def _build_nc():
    if "nc" in _CACHE:
        return _CACHE["nc"]
    import concourse.bacc as bacc
    import concourse.tile as tile
    from concourse import mybir

    f32 = mybir.dt.float32
    f32r = mybir.dt.float32r
    bf16 = mybir.dt.bfloat16
    AX = mybir.AxisListType
    OP = mybir.AluOpType
    AF = mybir.ActivationFunctionType

    nc = bacc.Bacc("TRN2", num_devices=NCORES)
    xpad = nc.dram_tensor("xpad", (C, PW * PW), bf16, kind="ExternalInput")
    wconvk = nc.dram_tensor("wconvk", (128, NH * NMM * 128), bf16, kind="ExternalInput")
    wconvv = nc.dram_tensor("wconvv", (128, NH * NMM * 128), bf16, kind="ExternalInput")
    wconvvT = nc.dram_tensor("wconvvT", (128, 3 * NMM * 128), bf16, kind="ExternalInput")
    wq = nc.dram_tensor("wq", (128, 128), bf16, kind="ExternalInput")
    outwp = nc.dram_tensor("outwp", (128, 64), f32, kind="ExternalInput")
    stack2 = nc.dram_tensor("stack2", (64, 128), f32, kind="ExternalInput")
    ind4 = nc.dram_tensor("ind4", (128, 4), f32, kind="ExternalInput")
    ind4T = nc.dram_tensor("ind4T", (4, 128), f32, kind="ExternalInput")
    idb = nc.dram_tensor("idb", (128, 128), bf16, kind="ExternalInput")
    idr = nc.dram_tensor("idr", (64, 64), f32r, kind="ExternalInput")
    idf = nc.dram_tensor("idf", (128, 128), f32, kind="ExternalInput")
    gamma = nc.dram_tensor("gamma", (64, 1), f32, kind="ExternalInput")
    beta = nc.dram_tensor("beta", (64, 1), f32, kind="ExternalInput")
    yout = nc.dram_tensor("yout", (64, S), f32, kind="ExternalOutput")

    with tile.TileContext(nc) as tc:
        import contextlib
        stk = contextlib.ExitStack()
        consts = stk.enter_context(tc.tile_pool(name="consts", bufs=1))
        cenp = stk.enter_context(tc.tile_pool(name="cenp", bufs=1))
        qp = stk.enter_context(tc.tile_pool(name="qp", bufs=1))
        stage = stk.enter_context(tc.tile_pool(name="stage", bufs=3))
        statp = stk.enter_context(tc.tile_pool(name="statp", bufs=2))
        smallp = stk.enter_context(tc.tile_pool(name="smallp", bufs=1))
        dramp = stk.enter_context(tc.tile_pool(name="dramp", bufs=1, space="DRAM"))
        kinv_d = dramp.tile([1, 512], f32)
        cc_in = dramp.tile([64, 2], f32)
        cc_out = dramp.tile([NCORES * 64, 2], f32, addr_space="Shared")

        # ---- constants (critical-path loads first) ----
        wq_sb = consts.tile([128, 128], bf16)
        nc.gpsimd.dma_start(out=wq_sb, in_=wq[:, :])
        # ---- cen tiles: T* = vertical-shift upper, TH* = horizontal-shift upper
        T0 = cenp.tile([128, TROWS * PW], bf16, name="T0")
        T1 = cenp.tile([128, TROWS * PW], bf16, name="T1")
        TH0 = cenp.tile([128, TROWS * PW], bf16, name="TH0")
        TH1 = cenp.tile([128, TROWS * PW], bf16, name="TH1")
        Ts = [T0, T1]
        THs = [TH0, TH1]
        HTP = TROWS * PW // 2
        nc.gpsimd.dma_start(out=T0[0:64, 0:HTP], in_=xpad[:, 0:HTP])
        nc.gpsimd.dma_start(out=T0[0:64, HTP:2 * HTP], in_=xpad[:, HTP:2 * HTP])

        idb_sb = consts.tile([128, 128], bf16)
        nc.gpsimd.dma_start(out=idb_sb, in_=idb[:, :])
        wck_sb = consts.tile([128, NH * NMM * 128], bf16)
        wcv_sb = consts.tile([128, NH * NMM * 128], bf16)
        wcvT_sb = consts.tile([128, 3 * NMM * 128], bf16)
        wck_v = wck_sb.rearrange("p (b c o) -> p b c o", b=NH, c=NMM)
        wcv_v = wcv_sb.rearrange("p (b c o) -> p b c o", b=NH, c=NMM)
        wcvT_v = wcvT_sb.rearrange("p (b c o) -> p b c o", b=3, c=NMM)
        outwp_sb = consts.tile([128, 64], f32)
        stack2_sb = consts.tile([64, 128], f32)
        ind4_sb = consts.tile([128, 4], f32)
        ind4T_sb = consts.tile([4, 128], f32)
        idr_sb = consts.tile([64, 64], f32r)
        idf_sb = consts.tile([128, 128], f32)
        gamma_sb = consts.tile([64, 1], f32)
        beta_sb = consts.tile([64, 1], f32)
        eps_in = consts.tile([128, 1], f32)
        nc.vector.memset(eps_in, EPS)
        eps_tiny = consts.tile([128, 1], f32)
        nc.vector.memset(eps_tiny, 1e-30)
        allones = consts.tile([128, 128], f32)
        nc.vector.memset(allones, 1.0)

        deferred = []

        def load_rest_of_consts():
            deferred.append(nc.gpsimd.dma_start(out=wcv_sb, in_=wconvv[:, :]))
            deferred.append(nc.gpsimd.dma_start(out=wcvT_sb, in_=wconvvT[:, :]))
            deferred.append(nc.gpsimd.dma_start(out=outwp_sb, in_=outwp[:, :]))
            deferred.append(nc.gpsimd.dma_start(out=stack2_sb, in_=stack2[:, :]))
            deferred.append(nc.gpsimd.dma_start(out=ind4_sb, in_=ind4[:, :]))
            deferred.append(nc.gpsimd.dma_start(out=ind4T_sb, in_=ind4T[:, :]))
            deferred.append(nc.gpsimd.dma_start(out=idr_sb, in_=idr[:, :]))
            deferred.append(nc.gpsimd.dma_start(out=idf_sb, in_=idf[:, :]))
            deferred.append(nc.gpsimd.dma_start(out=gamma_sb, in_=gamma[:, :]))
            deferred.append(nc.gpsimd.dma_start(out=beta_sb, in_=beta[:, :]))


        def ecopy(idx, out, in_):
            if idx % 2:
                nc.vector.tensor_copy(out, in_)
            else:
                nc.scalar.copy(out=out, in_=in_)

        def rebuild_upper(Tt, d):
            n = min(TROWS * PW, PW * PW - 2 * d * PW)
            nc.gpsimd.dma_start(out=Tt[64:128, 0:n],
                                in_=xpad[:, 2 * d * PW:2 * d * PW + n])

        def rebuild_upper_h(THt, d):
            n = TROWS * PW
            nc.gpsimd.dma_start(out=THt[64:128, 0:n],
                                in_=xpad[:, 2 * d:2 * d + n])

        def conv_taps(j, d, Tt, THt):
            tv = Tt.rearrange("p (h w) -> p h w", h=TROWS)
            thv = THt.rearrange("p (h w) -> p h w", h=TROWS)
            y0 = 4 * j
            out = []
            for m in range(NMM):
                if m < 3:
                    src, r0, c0 = tv, PAD + y0 - d, PAD + (m - 1) * d
                elif m == 3:
                    src, r0, c0 = thv, PAD + y0, PAD - d
                else:
                    src, r0, c0 = thv, PAD + y0, PAD
                out.append(src[:, r0:r0 + 4, c0:c0 + W])
            return out

        def conv_chunk(psum, kv, i, d, j, Tt, THt):
            wv = wck_v if kv == 0 else wcv_v
            for m, src in enumerate(conv_taps(j, d, Tt, THt)):
                nc.tensor.matmul(psum, wv[:, i, m, :], src,
                                 start=(m == 0), stop=(m == NMM - 1))

        # ---- big persistent tiles ----
        QT = qp.tile([128, S], bf16, tag="qt")     # [s%128, (jj, q2)] view
        QT_v = QT.rearrange("p (j q) -> p j q", j=72)
        kns = smallp.tile([128, 4], f32)
        scores_sb = smallp.tile([128, 4, 128], f32)
        attn_sb = smallp.tile([128, 4, 128], f32)
        sq_sb = smallp.tile([128, 4, 128], f32)
        rs = smallp.tile([128, 2], f32)
        mu41 = smallp.tile([4, 1], f32)
        e241 = smallp.tile([4, 1], f32)
        var41 = smallp.tile([4, 1], f32)
        rstd41 = smallp.tile([4, 1], f32)
        mx = smallp.tile([128, 1], f32)
        sm = smallp.tile([128, 1], f32)
        CT_sb = smallp.tile([128, 4, 64], bf16)
        DT_sb = smallp.tile([128, 3, NMM, 64], bf16)
        mu_bc = smallp.tile([128, 1], f32)
        rstd_bc = smallp.tile([128, 1], f32)

        # ---- PE warmup: ramp the clock while the image DMA lands ----
        with tc.tile_pool(name="warmp", bufs=2, space="PSUM") as warmp:
            for w in range(56):
                wp = warmp.tile([128, 64], f32, tag="warm", name="wp")
                nc.tensor.matmul(wp, wq_sb, wq_sb[:, 0:64], start=True, stop=True)

        # ================= Phase A + K phase =================
        with tc.tile_pool(name="convp", bufs=3, space="PSUM") as convp, \
             tc.tile_pool(name="tpp", bufs=2, space="PSUM") as tpp, \
             tc.tile_pool(name="scp", bufs=1, space="PSUM") as scp:
            sA = scp.tile([128, 256], f32, name="sA")
            sB = scp.tile([128, 256], f32, name="sB")

            # Q conv (K=64: no upper-half dependency), streamed in chunks
            t0v = T0.rearrange("p (h w) -> p h w", h=TROWS)
            qstats = smallp.tile([128, NCH, 6], f32)
            for j in range(NCH):
                pq = convp.tile([128, NT], f32, tag="conv", name="pq")
                nc.tensor.matmul(pq, wq_sb[0:64, :],
                                 t0v[0:64, PAD + 4 * j:PAD + 4 * j + 4, PAD:PAD + W],
                                 start=True, stop=True)
                qc = stage.tile([128, NT], bf16, tag="kc", name="qc")
                ecopy(j, qc, pq)
                nc.vector.bn_stats(out=qstats[:, j, :], in_=qc)
                for c3 in range(3):
                    tq = tpp.tile([128, 128], bf16, tag="tp", name="tq")
                    nc.tensor.transpose(tq, qc[:, 128 * c3:128 * (c3 + 1)], idb_sb)
                    ecopy(c3, QT_v[:, 3 * j + c3, :], tq)

            rebuild_upper(T0, SHIFTS[0])
            rebuild_upper_h(TH0, SHIFTS[0])
            nc.gpsimd.dma_start(out=TH0[0:64, :], in_=xpad[:, 0:TROWS * PW])
            nc.gpsimd.dma_start(out=wck_sb, in_=wconvk[:, :])
            nc.gpsimd.dma_start(out=T1[0:64, :], in_=xpad[:, 0:TROWS * PW])
            nc.gpsimd.dma_start(out=TH1[0:64, :], in_=xpad[:, 0:TROWS * PW])
            rebuild_upper(T1, SHIFTS[1])
            rebuild_upper_h(TH1, SHIFTS[1])
            load_rest_of_consts()

            qaggr = smallp.tile([128, 2], f32)
            nc.vector.bn_aggr(out=qaggr, in_=qstats)
            qinv = smallp.tile([128, 1], f32)
            nc.vector.tensor_mul(qinv, qaggr[:, 0:1], qaggr[:, 0:1])
            nc.vector.tensor_add(qinv, qinv, qaggr[:, 1:2])
            nc.scalar.mul(qinv, qinv, float(S))
            nc.scalar.activation(out=qinv, in_=qinv, func=AF.Sqrt,
                                 bias=eps_tiny, scale=1.0)
            nc.vector.reciprocal(out=qinv, in_=qinv)
            nc.scalar.mul(qinv, qinv, 1.0 / float(np.sqrt(np.float32(S))))

            first_score = [True]
            for it in range(4):
                i, d = it, SHIFTS[it]
                Tt, THt = Ts[it % 2], THs[it % 2]
                if it >= 1 and it + 1 < 4:
                    # prefetch shift it+1's uppers (WAR on shift it-1 just cleared)
                    rebuild_upper(Ts[(it + 1) % 2], SHIFTS[it + 1])
                    rebuild_upper_h(THs[(it + 1) % 2], SHIFTS[it + 1])
                kstats = statp.tile([128, NCH, 6], f32, tag="kstats")
                for j in range(NCH):
                    pc = convp.tile([128, NT], f32, tag="conv", name="pc")
                    conv_chunk(pc, 0, i, d, j, Tt, THt)
                    kc = stage.tile([128, NT], bf16, tag="kc")
                    ecopy(j, kc, pc)
                    nc.vector.bn_stats(out=kstats[:, j, :], in_=kc)
                    for c3 in range(3):
                        tp = tpp.tile([128, 128], bf16, tag="tp", name="tp")
                        nc.tensor.transpose(tp, kc[:, 128 * c3:128 * (c3 + 1)], idb_sb)
                        ktc = stage.tile([128, 128], bf16, tag="ktc")
                        ecopy(c3 + 1, ktc, tp)
                        jj = 3 * j + c3
                        psc = sA if i < 2 else sB
                        nc.tensor.matmul(psc[:, 128 * (i % 2):128 * (i % 2 + 1)],
                                         QT_v[:, jj, :], ktc,
                                         start=first_score[0], stop=False,
                                         skip_group_check=True)
                        first_score[0] = False
                kaggr = statp.tile([128, 2], f32, tag="kaggr")
                nc.vector.bn_aggr(out=kaggr, in_=kstats)
                nc.vector.tensor_mul(kns[:, i:i + 1], kaggr[:, 0:1], kaggr[:, 0:1])
                nc.vector.tensor_add(kns[:, i:i + 1], kns[:, i:i + 1], kaggr[:, 1:2])
                nc.scalar.mul(kns[:, i:i + 1], kns[:, i:i + 1], float(S))

            # kinv = rsqrt(kns); flatten to free dim via PE transpose, then
            # broadcast to all partitions with a ones-lhsT matmul (no DRAM hop)
            kinv = smallp.tile([128, 4], f32)
            nc.scalar.activation(out=kinv, in_=kns, func=AF.Sqrt,
                                 bias=eps_tiny, scale=1.0)
            nc.vector.reciprocal(out=kinv, in_=kinv)
            # flatten kinv to one partition row via PE transpose + a tracked
            # DRAM hop, then broadcast to all partitions with a K=1 matmul
            tkv = tpp.tile([128, 128], f32, tag="tp", name="tkv")
            nc.tensor.transpose(tkv[0:4, :], kinv, idf_sb)
            kinvT = smallp.tile([4, 128], f32)
            nc.vector.tensor_copy(kinvT, tkv[0:4, :])
            nc.sync.dma_start(out=kinv_d[0:1, :].rearrange("a (i o) -> a i o", i=4)[0],
                              in_=kinvT)
            krow = smallp.tile([1, 512], f32)
            nc.sync.dma_start(out=krow, in_=kinv_d[0:1, :])
            kbc = scp.tile([128, 512], f32, name="kbc")
            nc.tensor.matmul(kbc, allones[0:1, :], krow,
                             start=True, stop=True)

            # evict scores with qinv row scaling
            nc.vector.tensor_scalar(out=scores_sb[:, 0:2, :], in0=sA,
                                    scalar1=qinv, scalar2=None,
                                    op0=OP.mult)
            nc.vector.tensor_scalar(out=scores_sb[:, 2:4, :], in0=sB,
                                    scalar1=qinv, scalar2=None,
                                    op0=OP.mult)
            nc.vector.tensor_mul(scores_sb, scores_sb, kbc.rearrange(
                "p (i o) -> p i o", i=4))
            nc.scalar.activation(out=sq_sb, in_=scores_sb, func=AF.Square)
            for g in range(4):
                p0 = 32 * g
                blk = scores_sb[p0:p0 + 16, :, p0:p0 + 32]
                sqb = sq_sb[p0:p0 + 16, :, p0:p0 + 32]
                nc.vector.tensor_reduce(out=rs[p0:p0 + 16, 0:1], in_=blk,
                                        axis=AX.XY, op=OP.add)
                nc.vector.tensor_reduce(out=rs[p0:p0 + 16, 1:2], in_=sqb,
                                        axis=AX.XY, op=OP.add)

        # ================= V phase =================
        # y2 layout: [128, 4608]; group g cols [384g, 384g+384):
        #   partitions 0:64  = y chunk 2g,  partitions 64:128 = y chunk 2g+1
        y2 = qp.tile([128, S // 2], f32, tag="qt", name="y2")
        ystats = smallp.tile([128, NG, 6], f32)
        with tc.tile_pool(name="convp2", bufs=2, space="PSUM") as convp2, \
             tc.tile_pool(name="yp", bufs=3, space="PSUM") as yp:

            def emit_C():
                # C_i = outwp.T @ attn_i ; then transpose -> CT_sb
                for i in range(4):
                    cp = yp.tile([64, 128], f32, tag="cp", bufs=1, name="cp")
                    nc.tensor.matmul(cp, outwp_sb, attn_sb[:, i, :],
                                     start=True, stop=True)
                    cr = smallp.tile([64, 128], f32r, name=f"cr{i}", tag="cr")
                    nc.vector.tensor_copy(cr, cp)
                    ctp = yp.tile([128, 64], f32r, tag="ctp", bufs=1, name="ctp")
                    nc.tensor.transpose(ctp, cr, idr_sb)
                    nc.vector.tensor_copy(CT_sb[:, i, :], ctp)

            def emit_DT():
                # DT_im = WvT_im @ C_i^T : V-conv lhsT with out_w+attn folded in
                for i in range(3):
                    for m in range(NMM):
                        dp = yp.tile([128, 64], f32, tag="dp", bufs=1, name="dp")
                        nc.tensor.matmul(dp, wcvT_v[:, i, m, :], CT_sb[:, i, :],
                                         start=True, stop=True)
                        nc.vector.tensor_copy(DT_sb[:, i, m, :], dp)

            def emit_mid2():
                # per-head partition reduce on the PE + mean/var/rstd
                sums4p = yp.tile([4, 2], f32, tag="dp", bufs=1, name="sums4p")
                nc.tensor.matmul(sums4p, ind4_sb, rs, start=True, stop=True)
                nc.vector.tensor_scalar(out=mu41, in0=sums4p[:, 0:1],
                                        scalar1=1.0 / 2048.0, scalar2=None,
                                        op0=OP.mult)
                nc.vector.tensor_scalar(out=e241, in0=sums4p[:, 1:2],
                                        scalar1=1.0 / 2048.0, scalar2=None,
                                        op0=OP.mult)
                nc.vector.tensor_mul(var41, mu41, mu41)
                nc.vector.tensor_sub(var41, e241, var41)
                nc.scalar.activation(out=rstd41, in_=var41, func=AF.Sqrt,
                                     bias=eps_in[0:4, :], scale=1.0)
                nc.vector.reciprocal(out=rstd41, in_=rstd41)

            def emit_mid3():
                # per-head mu/rstd broadcast via PE, then softmax
                mu_ps = yp.tile([128, 1], f32, tag="dp", bufs=1, name="mu_ps")
                nc.tensor.matmul(mu_ps, ind4T_sb, mu41, start=True, stop=True)
                nc.vector.tensor_copy(mu_bc, mu_ps)
                rstd_ps = yp.tile([128, 1], f32, tag="dp", bufs=1, name="rstd_ps")
                nc.tensor.matmul(rstd_ps, ind4T_sb, rstd41, start=True, stop=True)
                nc.vector.tensor_copy(rstd_bc, rstd_ps)
                nc.vector.memset(attn_sb, 0.0)
                for g in range(4):
                    p0 = 32 * g
                    blk = scores_sb[p0:p0 + 16, :, p0:p0 + 32]
                    nc.vector.tensor_scalar(out=blk, in0=blk,
                                            scalar1=mu_bc[p0:p0 + 16, :],
                                            scalar2=rstd_bc[p0:p0 + 16, :],
                                            op0=OP.subtract, op1=OP.mult)
                    nc.vector.tensor_reduce(out=mx[p0:p0 + 16, :], in_=blk,
                                            axis=AX.XY, op=OP.max)
                    nc.vector.tensor_scalar(out=blk, in0=blk,
                                            scalar1=mx[p0:p0 + 16, :],
                                            scalar2=None, op0=OP.subtract)
                    nc.scalar.activation(out=blk, in_=blk, func=AF.Exp)
                    nc.vector.tensor_reduce(out=sm[p0:p0 + 16, :], in_=blk,
                                            axis=AX.XY, op=OP.add)
                    nc.vector.reciprocal(out=sm[p0:p0 + 16, :],
                                         in_=sm[p0:p0 + 16, :])
                    nc.vector.tensor_scalar(
                        out=attn_sb[p0:p0 + 16, :, p0:p0 + 32], in0=blk,
                        scalar1=sm[p0:p0 + 16, :], scalar2=None, op0=OP.mult)
                # park the scalar activation table on Relu for the BN tail
                nc.scalar.activation(out=rs[0:1, 0:1], in_=rs[0:1, 0:1],
                                     func=AF.Relu)

            # ---- it2 = 0: d=8 unfused path (covers attention latency) ----
            i3, d3 = 3, SHIFTS[3]
            Tt, THt = Ts[1], THs[1]
            vcs = []

            def emit_pair(g):
                pt = yp.tile([128, NT], f32, tag="py", name="pt")
                nc.tensor.matmul(pt[0:64, :], CT_sb[:, 3, :], vcs[2 * g],
                                 start=True, stop=True, skip_group_check=True)
                nc.tensor.matmul(pt[64:128, :], CT_sb[:, 3, :], vcs[2 * g + 1],
                                 start=True, stop=True, skip_group_check=True)
                nc.vector.tensor_copy(y2[:, NT * g:NT * (g + 1)], pt)

            for j in range(NCH):
                pc = convp2.tile([128, NT], f32, tag="conv", name="pc2")
                conv_chunk(pc, 1, i3, d3, j, Tt, THt)
                vc = stage.tile([128, NT], bf16, tag="vc", bufs=26)
                ecopy(j, vc, pc)
                vcs.append(vc)
                if j == 5:
                    emit_mid2()
                elif j == 8:
                    emit_mid3()
                elif j == 18:
                    emit_C()
                    emit_DT()
                    for g in range(9):
                        emit_pair(g)
                elif j > 18 and j % 2 == 1:
                    emit_pair(j // 2)

            # ---- it2 = 1..3: D-folded shifts write y directly ----
            for it2 in range(1, 4):
                i = 3 - it2
                d = SHIFTS[i]
                Tt, THt = Ts[i % 2], THs[i % 2]
                if it2 + 1 < 4:
                    # prefetch next shift's uppers (WAR just cleared)
                    ni = 3 - (it2 + 1)
                    rebuild_upper(Ts[ni % 2], SHIFTS[ni])
                    rebuild_upper_h(THs[ni % 2], SHIFTS[ni])
                for g in range(NG):
                    pt = yp.tile([128, NT], f32, tag="py", name="pt2")
                    for half, j in ((0, 2 * g), (1, 2 * g + 1)):
                        pap = pt[0:64, :] if half == 0 else pt[64:128, :]
                        for m, src in enumerate(conv_taps(j, d, Tt, THt)):
                            nc.tensor.matmul(pap, DT_sb[:, i, m, :], src,
                                             start=(m == 0), stop=(m == NMM - 1),
                                             skip_group_check=True)
                    sl = slice(NT * g, NT * (g + 1))
                    nc.vector.tensor_add(y2[:, sl], y2[:, sl], pt)
                    if it2 == 3:
                        nc.vector.bn_stats(out=ystats[:, g, :], in_=y2[:, sl])

        # ================= BN tail =================
        with tc.tile_pool(name="tailp", bufs=1, space="PSUM") as tailp:
            yaggr = smallp.tile([128, 2], f32)
            nc.vector.bn_aggr(out=yaggr, in_=ystats)
            # per-partition linear stats: [mean, E[x^2]] over each 4608 half
            pstat = smallp.tile([128, 2], f32)
            nc.vector.tensor_copy(pstat[:, 0:1], yaggr[:, 0:1])
            m2y = smallp.tile([128, 1], f32)
            nc.vector.tensor_mul(m2y, yaggr[:, 0:1], yaggr[:, 0:1])
            nc.vector.tensor_add(pstat[:, 1:2], yaggr[:, 1:2], m2y)
            # fold halves: bn[c] = pstat[c] + pstat[c+64]  (PE with identity slices)
            pf = tailp.tile([64, 2], f32, name="pf")
            nc.tensor.matmul(pf, idf_sb[:, 0:64], pstat, start=True, stop=False)
            nc.tensor.matmul(pf, idf_sb[:, 64:128], pstat, start=False, stop=True)
            bnloc = smallp.tile([64, 2], f32)
            nc.vector.tensor_scalar(out=bnloc, in0=pf, scalar1=float(S // 2),
                                    scalar2=None, op0=OP.mult)
            nc.sync.dma_start(out=cc_in[:, :], in_=bnloc)
            nc.gpsimd.collective_compute(
                "AllReduce", mybir.AluOpType.add,
                replica_groups=[list(range(NCORES))],
                ins=[cc_in[:, :]], outs=[cc_out[0:64, :]])
            grs = smallp.tile([64, 2], f32)
            nc.sync.dma_start(out=grs, in_=cc_out[0:64, :])
            mom = smallp.tile([64, 2], f32)
            nc.vector.tensor_scalar(out=mom, in0=grs, scalar1=1.0 / (B * S),
                                    scalar2=None, op0=OP.mult)
            meang = mom[:, 0:1]
            varg = smallp.tile([64, 1], f32)
            nc.vector.tensor_mul(varg, meang, meang)
            nc.vector.tensor_sub(varg, mom[:, 1:2], varg)
            # rsqrt(var+eps) via vector ops: sqrt unavailable -> use scalar Sqrt
            scaleg = smallp.tile([64, 1], f32)
            nc.scalar.activation(out=scaleg, in_=varg, func=AF.Sqrt,
                                 bias=eps_in[0:64, :], scale=1.0)
            nc.vector.reciprocal(out=scaleg, in_=scaleg)
            nc.vector.tensor_mul(scaleg, scaleg, gamma_sb)
            shiftg = smallp.tile([64, 1], f32)
            nc.vector.tensor_mul(shiftg, meang, scaleg)
            nc.vector.tensor_sub(shiftg, beta_sb, shiftg)
            # broadcast scale/shift to both partition halves via stack2
            ssg = smallp.tile([64, 2], f32)
            nc.vector.tensor_copy(ssg[:, 0:1], scaleg)
            nc.vector.tensor_copy(ssg[:, 1:2], shiftg)
            ps2 = tailp.tile([128, 2], f32, name="ps2")
            nc.tensor.matmul(ps2, stack2_sb, ssg, start=True, stop=True)
            ss2 = smallp.tile([128, 2], f32)
            nc.vector.tensor_copy(ss2, ps2)
            # affine+ReLU per 1152-col slice, then one strided DMA per half
            yv2 = yout[:, :].rearrange("c (g two t) -> c g two t", two=2, t=NT)
            for q4 in range(4):
                sl = slice(1152 * q4, 1152 * (q4 + 1))
                nc.scalar.activation(out=y2[:, sl], in_=y2[:, sl], func=AF.Relu,
                                     bias=ss2[:, 1:2], scale=ss2[:, 0:1])
                ylo = y2[0:64, sl].rearrange("c (g t) -> c g t", t=NT)
                yhi = y2[64:128, sl].rearrange("c (g t) -> c g t", t=NT)
                nc.sync.dma_start(out=yv2[:, 3 * q4:3 * q4 + 3, 0, :], in_=ylo)
                nc.sync.dma_start(out=yv2[:, 3 * q4:3 * q4 + 3, 1, :], in_=yhi)
        stk.close()
    nc.compile()
    _CACHE["nc"] = nc
    return nc


# ---------------------------------------------------------------- entry point
def kernel(cen, sum_weights, q_w, k_w, v_w, out_w, bn_gamma, bn_beta):
    from concourse.bass_utils import run_bass_kernel_spmd
    import ml_dtypes
    cen = np.asarray(cen, np.float32)
    wconvk, wconvv, wcvT, wq, outwp, stack2, ind4, ind4T = _host_weights(
        np.asarray(sum_weights), np.asarray(q_w),
        np.asarray(k_w), np.asarray(v_w), np.asarray(out_w))
    idb = np.eye(128, dtype=ml_dtypes.bfloat16)
    idr = np.eye(64, dtype=np.float32)
    idf = np.eye(128, dtype=np.float32)
    gam = np.asarray(bn_gamma, np.float32).reshape(64, 1)
    bet = np.asarray(bn_beta, np.float32).reshape(64, 1)

    import time as _t
    _t0 = _t.time()
    nc = _build_nc()
    print(f"[kernel] build+compile: {_t.time() - _t0:.1f}s", flush=True)
    in_maps = []
    for b in range(B):
        xp = np.zeros((C, PW, PW), np.float32)
        xp[:, PAD:PAD + H, PAD:PAD + W] = cen[b]
        in_maps.append({
            "xpad": xp.reshape(C, PW * PW).astype(ml_dtypes.bfloat16),
            "wconvk": wconvk, "wconvv": wconvv, "wconvvT": wcvT, "wq": wq,
            "outwp": outwp, "stack2": stack2, "ind4": ind4, "ind4T": ind4T,
            "idb": idb, "idr": idr, "idf": idf,
            "gamma": gam, "beta": bet,
        })
    trace = bool(int(os.environ.get("KERNEL_TRACE", "0")))
    res = run_bass_kernel_spmd(nc, in_maps, core_ids=list(range(NCORES)),
                               trace=trace)
    kernel.last_exec_time_ns = res.exec_time_ns
    out = np.stack([res.results[b]["yout"].reshape(64, H, W) for b in range(B)])
    return out.astype(np.float32)


# revision 43
# speedup vs baseline: 1.0387x; 1.0387x over previous
"""Trainium2 Bass kernel for nn_ExpansionContrastModule.

Strategy (pure data-parallel, one batch per NeuronCore, 8 cores):
  - Fuse the depthwise contrast kernels + 1x1 k_w/v_w projections into
    dense per-shift 3x3 conv weights on the host (tiny). Image tiles are
    bf16; each conv is 5 K=128 matmuls per 4-row chunk: 3 "V-pair"
    matmuls (partitions 64-127 hold the padded image shifted down by
    2*d rows -> taps ky=-d/+d), 1 "H-pair" matmul (upper half shifted
    left by 2*d cols -> taps kx=-d/+d at ky=0), 1 solo (center tap).
  - K chunks are evicted as bf16, PE-transposed, and immediately folded
    into the attention score Gram matrix (contraction over all 9216
    spatial positions); K is never materialized in full.
  - L2 normalization of Q/K is folded into the tiny score matrix via
    row/col norm scalars. InstanceNorm + softmax on-chip.
  - V phase: out_w is folded into attention (C_i = out_w @ attn_i) and,
    for shifts d=1,2,4, C_i is further folded into the V conv weights on
    device (DT_im = WvT_im @ C_i^T), so those convs write y directly
    from 5 matmuls per chunk. The d=8 shift runs the unfused path first,
    keeping the PE busy while attention/C/DT compute.
  - y lives as [128, 4608] (chunk-interleaved halves) so accumulation,
    stats and ReLU use all 128 partitions.
  - Train-mode BatchNorm stats are AllReduced across the 8 cores.
Heads live at 32-aligned partition bases (rows 32g+q, q<16) so the
per-head stat matmuls satisfy base-partition rules.
"""
import os
os.environ.setdefault("JAX_PLATFORMS", "axon,cpu")
import numpy as np

SHIFTS = (1, 2, 4, 8)
NL = 8
B, C, H, W = 8, 64, 96, 96
NH, HID = 4, 16
S = H * W
PW, PAD = 112, 8
TROWS = 104             # padded rows kept on device (reads stay below 104)
NT = 384                 # conv chunk: 4 rows of 96
NCH = S // NT            # 24 chunks
NG = NCH // 2            # 12 paired chunk groups in the y layout
NMM = 5                  # matmuls per conv chunk (3 V-pairs, 1 H-pair, 1 solo)
NCORES = 8
EPS = 1e-5

# ---------------------------------------------------------------- host math
def _softmax(x, axis):
    m = x.max(axis=axis, keepdims=True)
    e = np.exp(x - m)
    return e / e.sum(axis=axis, keepdims=True)


def _host_weights(sum_weights, q_w, k_w, v_w, out_w):
    import ml_dtypes
    d1 = np.array([[[-1, 0, 0], [0, 1, 0], [0, 0, 0]],
                   [[0, -1, 0], [0, 1, 0], [0, 0, 0]],
                   [[0, 0, -1], [0, 1, 0], [0, 0, 0]],
                   [[0, 0, 0], [0, 1, -1], [0, 0, 0]]], dtype=np.float32).reshape(4, 1, 3, 3)
    d2 = d1[:, :, ::-1, ::-1].copy()
    K8 = np.concatenate([d1, d2], 0)[:, 0].astype(np.float64)   # [8,3,3]
    K0 = K8.mean(0)
    wconv = np.zeros((128, 2, NH, NMM, 128), np.float64)        # [part, kv, shift, mm, o]
    for i in range(NH):
        sw = _softmax(sum_weights[i].astype(np.float64), -1).reshape(C, 2)
        mixed = (K8[:, None] * sw[None, :, 0, None, None]
                 + K0[None, None] * sw[None, :, 1, None, None])  # [8,C,3,3]
        for kv, w in ((0, k_w), (1, v_w)):
            Weff = np.einsum('ojc,jcyx->ocyx',
                             w[i].astype(np.float64).reshape(128, NL, C), mixed)
            for m in range(3):      # V-pairs: kx index m, ky=0 lower / ky=2 upper
                wconv[0:64, kv, i, m, :] = Weff[:, :, 0, m].T
                wconv[64:128, kv, i, m, :] = Weff[:, :, 2, m].T
            # H-pair: ky=1 row, kx=-d lower / kx=+d upper
            wconv[0:64, kv, i, 3, :] = Weff[:, :, 1, 0].T
            wconv[64:128, kv, i, 3, :] = Weff[:, :, 1, 2].T
            # solo: center tap, upper zero
            wconv[0:64, kv, i, 4, :] = Weff[:, :, 1, 1].T
    bf16 = ml_dtypes.bfloat16
    # transposed V-conv weight blocks for the on-device D-fold (shifts 0..2)
    wcvT = np.zeros((128, 3, NMM, 128), np.float64)
    for i in range(3):
        for m in range(NMM):
            wcvT[:, i, m, :] = wconv[:, 1, i, m, :].T
    wq = np.zeros((128, 128), np.float64)        # lhsT [c2, q2]; rows>=64 zero
    for f in range(64):
        g, q = divmod(f, 16)
        wq[0:C, 32 * g + q] = q_w[f % 4, f // 4]
    outwp = np.zeros((128, 64), np.float32)      # lhsT rows c2=32g+q -> out_w[:, g*16+q]
    for c in range(64):
        g, q = divmod(c, 16)
        outwp[32 * g + q, :] = out_w[:, c]
    stack2 = np.zeros((64, 128), np.float32)     # lhsT: broadcast [64] -> [128] halves
    for c in range(64):
        stack2[c, c] = 1.0
        stack2[c, 64 + c] = 1.0
    ind4 = np.zeros((128, 4), np.float32)        # lhsT: per-head partition reduce
    ind4T = np.zeros((4, 128), np.float32)       # lhsT: per-head partition broadcast
    for g in range(4):
        ind4[32 * g:32 * g + 16, g] = 1.0
        ind4T[g, 32 * g:32 * g + 16] = 1.0
    return (wconv[:, 0].reshape(128, NH * NMM * 128).astype(bf16),
            wconv[:, 1].reshape(128, NH * NMM * 128).astype(bf16),
            wcvT.reshape(128, 3 * NMM * 128).astype(bf16),
            wq.astype(bf16), outwp, stack2, ind4, ind4T)


# ---------------------------------------------------------------- device build
_CACHE = {}


def _build_nc():
    if "nc" in _CACHE:
        return _CACHE["nc"]
    import concourse.bacc as bacc
    import concourse.tile as tile
    from concourse import mybir

    f32 = mybir.dt.float32
    f32r = mybir.dt.float32r
    bf16 = mybir.dt.bfloat16
    AX = mybir.AxisListType
    OP = mybir.AluOpType
    AF = mybir.ActivationFunctionType

    nc = bacc.Bacc("TRN2", num_devices=NCORES)
    xpad = nc.dram_tensor("xpad", (C, PW * PW), bf16, kind="ExternalInput")
    wconvk = nc.dram_tensor("wconvk", (128, NH * NMM * 128), bf16, kind="ExternalInput")
    wconvv = nc.dram_tensor("wconvv", (128, NH * NMM * 128), bf16, kind="ExternalInput")
    wconvvT = nc.dram_tensor("wconvvT", (128, 3 * NMM * 128), bf16, kind="ExternalInput")
    wq = nc.dram_tensor("wq", (128, 128), bf16, kind="ExternalInput")
    outwp = nc.dram_tensor("outwp", (128, 64), f32, kind="ExternalInput")
    stack2 = nc.dram_tensor("stack2", (64, 128), f32, kind="ExternalInput")
    ind4 = nc.dram_tensor("ind4", (128, 4), f32, kind="ExternalInput")
    ind4T = nc.dram_tensor("ind4T", (4, 128), f32, kind="ExternalInput")
    idb = nc.dram_tensor("idb", (128, 128), bf16, kind="ExternalInput")
    idr = nc.dram_tensor("idr", (64, 64), f32r, kind="ExternalInput")
    idf = nc.dram_tensor("idf", (128, 128), f32, kind="ExternalInput")
    gamma = nc.dram_tensor("gamma", (64, 1), f32, kind="ExternalInput")
    beta = nc.dram_tensor("beta", (64, 1), f32, kind="ExternalInput")
    yout = nc.dram_tensor("yout", (64, S), f32, kind="ExternalOutput")

    with tile.TileContext(nc) as tc:
        import contextlib
        stk = contextlib.ExitStack()
        consts = stk.enter_context(tc.tile_pool(name="consts", bufs=1))
        cenp = stk.enter_context(tc.tile_pool(name="cenp", bufs=1))
        qp = stk.enter_context(tc.tile_pool(name="qp", bufs=1))
        stage = stk.enter_context(tc.tile_pool(name="stage", bufs=3))
        statp = stk.enter_context(tc.tile_pool(name="statp", bufs=2))
        smallp = stk.enter_context(tc.tile_pool(name="smallp", bufs=1))
        dramp = stk.enter_context(tc.tile_pool(name="dramp", bufs=1, space="DRAM"))
        kinv_d = dramp.tile([1, 512], f32)
        cc_in = dramp.tile([64, 2], f32)
        cc_out = dramp.tile([NCORES * 64, 2], f32, addr_space="Shared")

        # ---- constants (critical-path loads first) ----
        wq_sb = consts.tile([128, 128], bf16)
        nc.gpsimd.dma_start(out=wq_sb, in_=wq[:, :])
        # ---- cen tiles: T* = vertical-shift upper, TH* = horizontal-shift upper
        T0 = cenp.tile([128, TROWS * PW], bf16, name="T0")
        T1 = cenp.tile([128, TROWS * PW], bf16, name="T1")
        TH0 = cenp.tile([128, TROWS * PW], bf16, name="TH0")
        TH1 = cenp.tile([128, TROWS * PW], bf16, name="TH1")
        Ts = [T0, T1]
        THs = [TH0, TH1]
        HTP = TROWS * PW // 2
        nc.gpsimd.dma_start(out=T0[0:64, 0:HTP], in_=xpad[:, 0:HTP])
        nc.gpsimd.dma_start(out=T0[0:64, HTP:2 * HTP], in_=xpad[:, HTP:2 * HTP])

        idb_sb = consts.tile([128, 128], bf16)
        nc.gpsimd.dma_start(out=idb_sb, in_=idb[:, :])
        wck_sb = consts.tile([128, NH * NMM * 128], bf16)
        wcv_sb = consts.tile([128, NH * NMM * 128], bf16)
        wcvT_sb = consts.tile([128, 3 * NMM * 128], bf16)
        wck_v = wck_sb.rearrange("p (b c o) -> p b c o", b=NH, c=NMM)
        wcv_v = wcv_sb.rearrange("p (b c o) -> p b c o", b=NH, c=NMM)
        wcvT_v = wcvT_sb.rearrange("p (b c o) -> p b c o", b=3, c=NMM)
        outwp_sb = consts.tile([128, 64], f32)
        stack2_sb = consts.tile([64, 128], f32)
        ind4_sb = consts.tile([128, 4], f32)
        ind4T_sb = consts.tile([4, 128], f32)
        idr_sb = consts.tile([64, 64], f32r)
        idf_sb = consts.tile([128, 128], f32)
        gamma_sb = consts.tile([64, 1], f32)
        beta_sb = consts.tile([64, 1], f32)
        eps_in = consts.tile([128, 1], f32)
        nc.vector.memset(eps_in, EPS)
        eps_tiny = consts.tile([128, 1], f32)
        nc.vector.memset(eps_tiny, 1e-30)
        allones = consts.tile([128, 128], f32)
        nc.vector.memset(allones, 1.0)

        deferred = []

        def load_rest_of_consts():
            deferred.append(nc.gpsimd.dma_start(out=wcv_sb, in_=wconvv[:, :]))
            deferred.append(nc.gpsimd.dma_start(out=wcvT_sb, in_=wconvvT[:, :]))
            deferred.append(nc.gpsimd.dma_start(out=outwp_sb, in_=outwp[:, :]))
            deferred.append(nc.gpsimd.dma_start(out=stack2_sb, in_=stack2[:, :]))
            deferred.append(nc.gpsimd.dma_start(out=ind4_sb, in_=ind4[:, :]))
            deferred.append(nc.gpsimd.dma_start(out=ind4T_sb, in_=ind4T[:, :]))
            deferred.append(nc.gpsimd.dma_start(out=idr_sb, in_=idr[:, :]))
            deferred.append(nc.gpsimd.dma_start(out=idf_sb, in_=idf[:, :]))
            deferred.append(nc.gpsimd.dma_start(out=gamma_sb, in_=gamma[:, :]))
            deferred.append(nc.gpsimd.dma_start(out=beta_sb, in_=beta[:, :]))


        def ecopy(idx, out, in_):
            if idx % 2:
                nc.vector.tensor_copy(out, in_)
            else:
                nc.scalar.copy(out=out, in_=in_)

        def rebuild_upper(Tt, d):
            n = min(TROWS * PW, PW * PW - 2 * d * PW)
            nc.gpsimd.dma_start(out=Tt[64:128, 0:n],
                                in_=xpad[:, 2 * d * PW:2 * d * PW + n])

        def rebuild_upper_h(THt, d):
            n = TROWS * PW
            nc.gpsimd.dma_start(out=THt[64:128, 0:n],
                                in_=xpad[:, 2 * d:2 * d + n])

        def conv_taps(j, d, Tt, THt):
            tv = Tt.rearrange("p (h w) -> p h w", h=TROWS)
            thv = THt.rearrange("p (h w) -> p h w", h=TROWS)
            y0 = 4 * j
            out = []
            for m in range(NMM):
                if m < 3:
                    src, r0, c0 = tv, PAD + y0 - d, PAD + (m - 1) * d
                elif m == 3:
                    src, r0, c0 = thv, PAD + y0, PAD - d
                else:
                    src, r0, c0 = thv, PAD + y0, PAD
                out.append(src[:, r0:r0 + 4, c0:c0 + W])
            return out

        def conv_chunk(psum, kv, i, d, j, Tt, THt):
            wv = wck_v if kv == 0 else wcv_v
            for m, src in enumerate(conv_taps(j, d, Tt, THt)):
                nc.tensor.matmul(psum, wv[:, i, m, :], src,
                                 start=(m == 0), stop=(m == NMM - 1))

        # ---- big persistent tiles ----
        QT = qp.tile([128, S], bf16, tag="qt")     # [s%128, (jj, q2)] view
        QT_v = QT.rearrange("p (j q) -> p j q", j=72)
        kns = smallp.tile([128, 4], f32)
        scores_sb = smallp.tile([128, 4, 128], f32)
        attn_sb = smallp.tile([128, 4, 128], f32)
        sq_sb = smallp.tile([128, 4, 128], f32)
        rs = smallp.tile([128, 2], f32)
        mu41 = smallp.tile([4, 1], f32)
        e241 = smallp.tile([4, 1], f32)
        var41 = smallp.tile([4, 1], f32)
        rstd41 = smallp.tile([4, 1], f32)
        mx = smallp.tile([128, 1], f32)
        sm = smallp.tile([128, 1], f32)
        CT_sb = smallp.tile([128, 4, 64], bf16)
        DT_sb = smallp.tile([128, 3, NMM, 64], bf16)
        mu_bc = smallp.tile([128, 1], f32)
        rstd_bc = smallp.tile([128, 1], f32)

        # ---- PE warmup: ramp the clock while the image DMA lands ----
        with tc.tile_pool(name="warmp", bufs=2, space="PSUM") as warmp:
            for w in range(56):
                wp = warmp.tile([128, 64], f32, tag="warm", name="wp")
                nc.tensor.matmul(wp, wq_sb, wq_sb[:, 0:64], start=True, stop=True)

        # ================= Phase A + K phase =================
        with tc.tile_pool(name="convp", bufs=3, space="PSUM") as convp, \
             tc.tile_pool(name="tpp", bufs=2, space="PSUM") as tpp, \
             tc.tile_pool(name="scp", bufs=1, space="PSUM") as scp:
            sA = scp.tile([128, 256], f32, name="sA")
            sB = scp.tile([128, 256], f32, name="sB")

            # Q conv (K=64: no upper-half dependency), streamed in chunks
            t0v = T0.rearrange("p (h w) -> p h w", h=TROWS)
            qstats = smallp.tile([128, NCH, 6], f32)
            for j in range(NCH):
                pq = convp.tile([128, NT], f32, tag="conv", name="pq")
                nc.tensor.matmul(pq, wq_sb[0:64, :],
                                 t0v[0:64, PAD + 4 * j:PAD + 4 * j + 4, PAD:PAD + W],
                                 start=True, stop=True)
                qc = stage.tile([128, NT], bf16, tag="kc", name="qc")
                ecopy(j, qc, pq)
                nc.vector.bn_stats(out=qstats[:, j, :], in_=qc)
                for c3 in range(3):
                    tq = tpp.tile([128, 128], bf16, tag="tp", name="tq")
                    nc.tensor.transpose(tq, qc[:, 128 * c3:128 * (c3 + 1)], idb_sb)
                    ecopy(c3, QT_v[:, 3 * j + c3, :], tq)

            rebuild_upper(T0, SHIFTS[0])
            rebuild_upper_h(TH0, SHIFTS[0])
            nc.gpsimd.dma_start(out=TH0[0:64, :], in_=xpad[:, 0:TROWS * PW])
            nc.gpsimd.dma_start(out=wck_sb, in_=wconvk[:, :])
            nc.gpsimd.dma_start(out=T1[0:64, :], in_=xpad[:, 0:TROWS * PW])
            nc.gpsimd.dma_start(out=TH1[0:64, :], in_=xpad[:, 0:TROWS * PW])
            rebuild_upper(T1, SHIFTS[1])
            rebuild_upper_h(TH1, SHIFTS[1])
            load_rest_of_consts()

            qaggr = smallp.tile([128, 2], f32)
            nc.vector.bn_aggr(out=qaggr, in_=qstats)
            qinv = smallp.tile([128, 1], f32)
            nc.vector.tensor_mul(qinv, qaggr[:, 0:1], qaggr[:, 0:1])
            nc.vector.tensor_add(qinv, qinv, qaggr[:, 1:2])
            nc.scalar.mul(qinv, qinv, float(S))
            nc.scalar.activation(out=qinv, in_=qinv, func=AF.Sqrt,
                                 bias=eps_tiny, scale=1.0)
            nc.vector.reciprocal(out=qinv, in_=qinv)
            nc.scalar.mul(qinv, qinv, 1.0 / float(np.sqrt(np.float32(S))))

            first_score = [True]
            for it in range(4):
                i, d = it, SHIFTS[it]
                Tt, THt = Ts[it % 2], THs[it % 2]
                if it >= 1 and it + 1 < 4:
                    # prefetch shift it+1's uppers (WAR on shift it-1 just cleared)
                    rebuild_upper(Ts[(it + 1) % 2], SHIFTS[it + 1])
                    rebuild_upper_h(THs[(it + 1) % 2], SHIFTS[it + 1])
                kstats = statp.tile([128, NCH, 6], f32, tag="kstats")
                for j in range(NCH):
                    pc = convp.tile([128, NT], f32, tag="conv", name="pc")
                    conv_chunk(pc, 0, i, d, j, Tt, THt)
                    kc = stage.tile([128, NT], bf16, tag="kc")
                    ecopy(j, kc, pc)
                    nc.vector.bn_stats(out=kstats[:, j, :], in_=kc)
                    for c3 in range(3):
                        tp = tpp.tile([128, 128], bf16, tag="tp", name="tp")
                        nc.tensor.transpose(tp, kc[:, 128 * c3:128 * (c3 + 1)], idb_sb)
                        ktc = stage.tile([128, 128], bf16, tag="ktc")
                        ecopy(c3 + 1, ktc, tp)
                        jj = 3 * j + c3
                        psc = sA if i < 2 else sB
                        nc.tensor.matmul(psc[:, 128 * (i % 2):128 * (i % 2 + 1)],
                                         QT_v[:, jj, :], ktc,
                                         start=first_score[0], stop=False,
                                         skip_group_check=True)
                        first_score[0] = False
                kaggr = statp.tile([128, 2], f32, tag="kaggr")
                nc.vector.bn_aggr(out=kaggr, in_=kstats)
                nc.vector.tensor_mul(kns[:, i:i + 1], kaggr[:, 0:1], kaggr[:, 0:1])
                nc.vector.tensor_add(kns[:, i:i + 1], kns[:, i:i + 1], kaggr[:, 1:2])
                nc.scalar.mul(kns[:, i:i + 1], kns[:, i:i + 1], float(S))

            # kinv = rsqrt(kns); flatten to free dim via PE transpose, then
            # broadcast to all partitions with a ones-lhsT matmul (no DRAM hop)
            kinv = smallp.tile([128, 4], f32)
            nc.scalar.activation(out=kinv, in_=kns, func=AF.Sqrt,
                                 bias=eps_tiny, scale=1.0)
            nc.vector.reciprocal(out=kinv, in_=kinv)
            # flatten kinv to one partition row via PE transpose + a tracked
            # DRAM hop, then broadcast to all partitions with a K=1 matmul
            tkv = tpp.tile([128, 128], f32, tag="tp", name="tkv")
            nc.tensor.transpose(tkv[0:4, :], kinv, idf_sb)
            kinvT = smallp.tile([4, 128], f32)
            nc.vector.tensor_copy(kinvT, tkv[0:4, :])
            nc.sync.dma_start(out=kinv_d[0:1, :].rearrange("a (i o) -> a i o", i=4)[0],
                              in_=kinvT)
            krow = smallp.tile([1, 512], f32)
            nc.sync.dma_start(out=krow, in_=kinv_d[0:1, :])
            kbc = scp.tile([128, 512], f32, name="kbc")
            nc.tensor.matmul(kbc, allones[0:1, :], krow,
                             start=True, stop=True)

            # evict scores with qinv row scaling
            nc.vector.tensor_scalar(out=scores_sb[:, 0:2, :], in0=sA,
                                    scalar1=qinv, scalar2=None,
                                    op0=OP.mult)
            nc.vector.tensor_scalar(out=scores_sb[:, 2:4, :], in0=sB,
                                    scalar1=qinv, scalar2=None,
                                    op0=OP.mult)
            nc.vector.tensor_mul(scores_sb, scores_sb, kbc.rearrange(
                "p (i o) -> p i o", i=4))
            nc.scalar.activation(out=sq_sb, in_=scores_sb, func=AF.Square)
            for g in range(4):
                p0 = 32 * g
                blk = scores_sb[p0:p0 + 16, :, p0:p0 + 32]
                sqb = sq_sb[p0:p0 + 16, :, p0:p0 + 32]
                nc.vector.tensor_reduce(out=rs[p0:p0 + 16, 0:1], in_=blk,
                                        axis=AX.XY, op=OP.add)
                nc.vector.tensor_reduce(out=rs[p0:p0 + 16, 1:2], in_=sqb,
                                        axis=AX.XY, op=OP.add)

        # ================= V phase =================
        # y2 layout: [128, 4608]; group g cols [384g, 384g+384):
        #   partitions 0:64  = y chunk 2g,  partitions 64:128 = y chunk 2g+1
        y2 = qp.tile([128, S // 2], f32, tag="qt", name="y2")
        ystats = smallp.tile([128, NG, 6], f32)
        with tc.tile_pool(name="convp2", bufs=2, space="PSUM") as convp2, \
             tc.tile_pool(name="yp", bufs=3, space="PSUM") as yp:

            def emit_C():
                # C_i = outwp.T @ attn_i ; then transpose -> CT_sb
                for i in range(4):
                    cp = yp.tile([64, 128], f32, tag="cp", bufs=1, name="cp")
                    nc.tensor.matmul(cp, outwp_sb, attn_sb[:, i, :],
                                     start=True, stop=True)
                    cr = smallp.tile([64, 128], f32r, name=f"cr{i}", tag="cr")
                    nc.vector.tensor_copy(cr, cp)
                    ctp = yp.tile([128, 64], f32r, tag="ctp", bufs=1, name="ctp")
                    nc.tensor.transpose(ctp, cr, idr_sb)
                    nc.vector.tensor_copy(CT_sb[:, i, :], ctp)

            def emit_DT():
                # DT_im = WvT_im @ C_i^T : V-conv lhsT with out_w+attn folded in
                for i in range(3):
                    for m in range(NMM):
                        dp = yp.tile([128, 64], f32, tag="dp", bufs=1, name="dp")
                        nc.tensor.matmul(dp, wcvT_v[:, i, m, :], CT_sb[:, i, :],
                                         start=True, stop=True)
                        nc.vector.tensor_copy(DT_sb[:, i, m, :], dp)

            def emit_mid2():
                # per-head partition reduce on the PE + mean/var/rstd
                sums4p = yp.tile([4, 2], f32, tag="dp", bufs=1, name="sums4p")
                nc.tensor.matmul(sums4p, ind4_sb, rs, start=True, stop=True)
                nc.vector.tensor_scalar(out=mu41, in0=sums4p[:, 0:1],
                                        scalar1=1.0 / 2048.0, scalar2=None,
                                        op0=OP.mult)
                nc.vector.tensor_scalar(out=e241, in0=sums4p[:, 1:2],
                                        scalar1=1.0 / 2048.0, scalar2=None,
                                        op0=OP.mult)
                nc.vector.tensor_mul(var41, mu41, mu41)
                nc.vector.tensor_sub(var41, e241, var41)
                nc.scalar.activation(out=rstd41, in_=var41, func=AF.Sqrt,
                                     bias=eps_in[0:4, :], scale=1.0)
                nc.vector.reciprocal(out=rstd41, in_=rstd41)

            def emit_mid3():
                # per-head mu/rstd broadcast via PE, then softmax
                mu_ps = yp.tile([128, 1], f32, tag="dp", bufs=1, name="mu_ps")
                nc.tensor.matmul(mu_ps, ind4T_sb, mu41, start=True, stop=True)
                nc.vector.tensor_copy(mu_bc, mu_ps)
                rstd_ps = yp.tile([128, 1], f32, tag="dp", bufs=1, name="rstd_ps")
                nc.tensor.matmul(rstd_ps, ind4T_sb, rstd41, start=True, stop=True)
                nc.vector.tensor_copy(rstd_bc, rstd_ps)
                nc.vector.memset(attn_sb, 0.0)
                for g in range(4):
                    p0 = 32 * g
                    blk = scores_sb[p0:p0 + 16, :, p0:p0 + 32]
                    nc.vector.tensor_scalar(out=blk, in0=blk,
                                            scalar1=mu_bc[p0:p0 + 16, :],
                                            scalar2=rstd_bc[p0:p0 + 16, :],
                                            op0=OP.subtract, op1=OP.mult)
                    nc.vector.tensor_reduce(out=mx[p0:p0 + 16, :], in_=blk,
                                            axis=AX.XY, op=OP.max)
                    nc.vector.tensor_scalar(out=blk, in0=blk,
                                            scalar1=mx[p0:p0 + 16, :],
                                            scalar2=None, op0=OP.subtract)
                    nc.scalar.activation(out=blk, in_=blk, func=AF.Exp)
                    nc.vector.tensor_reduce(out=sm[p0:p0 + 16, :], in_=blk,
                                            axis=AX.XY, op=OP.add)
                    nc.vector.reciprocal(out=sm[p0:p0 + 16, :],
                                         in_=sm[p0:p0 + 16, :])
                    nc.vector.tensor_scalar(
                        out=attn_sb[p0:p0 + 16, :, p0:p0 + 32], in0=blk,
                        scalar1=sm[p0:p0 + 16, :], scalar2=None, op0=OP.mult)
                # park the scalar activation table on Relu for the BN tail
                nc.scalar.activation(out=rs[0:1, 0:1], in_=rs[0:1, 0:1],
                                     func=AF.Relu)

            # ---- it2 = 0: d=8 unfused path (covers attention latency) ----
            i3, d3 = 3, SHIFTS[3]
            Tt, THt = Ts[1], THs[1]
            vcs = []

            def emit_pair(g):
                pt = yp.tile([128, NT], f32, tag="py", name="pt")
                nc.tensor.matmul(pt[0:64, :], CT_sb[:, 3, :], vcs[2 * g],
                                 start=True, stop=True, skip_group_check=True)
                nc.tensor.matmul(pt[64:128, :], CT_sb[:, 3, :], vcs[2 * g + 1],
                                 start=True, stop=True, skip_group_check=True)
                nc.vector.tensor_copy(y2[:, NT * g:NT * (g + 1)], pt)

            for j in range(NCH):
                pc = convp2.tile([128, NT], f32, tag="conv", name="pc2")
                conv_chunk(pc, 1, i3, d3, j, Tt, THt)
                vc = stage.tile([128, NT], bf16, tag="vc", bufs=26)
                ecopy(j, vc, pc)
                vcs.append(vc)
                if j == 5:
                    emit_mid2()
                elif j == 8:
                    emit_mid3()
                elif j == 18:
                    emit_C()
                    emit_DT()
                    for g in range(9):
                        emit_pair(g)
                elif j > 18 and j % 2 == 1:
                    emit_pair(j // 2)

            # ---- it2 = 1..3: D-folded shifts write y directly ----
            for it2 in range(1, 4):
                i = 3 - it2
                d = SHIFTS[i]
                Tt, THt = Ts[i % 2], THs[i % 2]
                if it2 + 1 < 4:
                    # prefetch next shift's uppers (WAR just cleared)
                    ni = 3 - (it2 + 1)
                    rebuild_upper(Ts[ni % 2], SHIFTS[ni])
                    rebuild_upper_h(THs[ni % 2], SHIFTS[ni])
                for g in range(NG):
                    pt = yp.tile([128, NT], f32, tag="py", name="pt2")
                    for half, j in ((0, 2 * g), (1, 2 * g + 1)):
                        pap = pt[0:64, :] if half == 0 else pt[64:128, :]
                        for m, src in enumerate(conv_taps(j, d, Tt, THt)):
                            nc.tensor.matmul(pap, DT_sb[:, i, m, :], src,
                                             start=(m == 0), stop=(m == NMM - 1),
                                             skip_group_check=True)
                    sl = slice(NT * g, NT * (g + 1))
                    nc.vector.tensor_add(y2[:, sl], y2[:, sl], pt)
                    if it2 == 3:
                        nc.vector.bn_stats(out=ystats[:, g, :], in_=y2[:, sl])

        # ================= BN tail =================
        with tc.tile_pool(name="tailp", bufs=1, space="PSUM") as tailp:
            yaggr = smallp.tile([128, 2], f32)
            nc.vector.bn_aggr(out=yaggr, in_=ystats)
            # per-partition linear stats: [mean, E[x^2]] over each 4608 half
            pstat = smallp.tile([128, 2], f32)
            nc.vector.tensor_copy(pstat[:, 0:1], yaggr[:, 0:1])
            m2y = smallp.tile([128, 1], f32)
            nc.vector.tensor_mul(m2y, yaggr[:, 0:1], yaggr[:, 0:1])
            nc.vector.tensor_add(pstat[:, 1:2], yaggr[:, 1:2], m2y)
            # fold halves: bn[c] = pstat[c] + pstat[c+64]  (PE with identity slices)
            pf = tailp.tile([64, 2], f32, name="pf")
            nc.tensor.matmul(pf, idf_sb[:, 0:64], pstat, start=True, stop=False)
            nc.tensor.matmul(pf, idf_sb[:, 64:128], pstat, start=False, stop=True)
            bnloc = smallp.tile([64, 2], f32)
            nc.vector.tensor_scalar(out=bnloc, in0=pf, scalar1=float(S // 2),
                                    scalar2=None, op0=OP.mult)
            nc.sync.dma_start(out=cc_in[:, :], in_=bnloc)
            nc.gpsimd.collective_compute(
                "AllReduce", mybir.AluOpType.add,
                replica_groups=[list(range(NCORES))],
                ins=[cc_in[:, :]], outs=[cc_out[0:64, :]])
            grs = smallp.tile([64, 2], f32)
            nc.sync.dma_start(out=grs, in_=cc_out[0:64, :])
            mom = smallp.tile([64, 2], f32)
            nc.vector.tensor_scalar(out=mom, in0=grs, scalar1=1.0 / (B * S),
                                    scalar2=None, op0=OP.mult)
            meang = mom[:, 0:1]
            varg = smallp.tile([64, 1], f32)
            nc.vector.tensor_mul(varg, meang, meang)
            nc.vector.tensor_sub(varg, mom[:, 1:2], varg)
            # rsqrt(var+eps) via vector ops: sqrt unavailable -> use scalar Sqrt
            scaleg = smallp.tile([64, 1], f32)
            nc.scalar.activation(out=scaleg, in_=varg, func=AF.Sqrt,
                                 bias=eps_in[0:64, :], scale=1.0)
            nc.vector.reciprocal(out=scaleg, in_=scaleg)
            nc.vector.tensor_mul(scaleg, scaleg, gamma_sb)
            shiftg = smallp.tile([64, 1], f32)
            nc.vector.tensor_mul(shiftg, meang, scaleg)
            nc.vector.tensor_sub(shiftg, beta_sb, shiftg)
            # broadcast scale/shift to both partition halves via stack2
            ssg = smallp.tile([64, 2], f32)
            nc.vector.tensor_copy(ssg[:, 0:1], scaleg)
            nc.vector.tensor_copy(ssg[:, 1:2], shiftg)
            ps2 = tailp.tile([128, 2], f32, name="ps2")
            nc.tensor.matmul(ps2, stack2_sb, ssg, start=True, stop=True)
            ss2 = smallp.tile([128, 2], f32)
            nc.vector.tensor_copy(ss2, ps2)
            # affine+ReLU per 1152-col slice, alternating scalar/vector so the
            # two engines overlap; one strided DMA per half per slice
            yv2 = yout[:, :].rearrange("c (g two t) -> c g two t", two=2, t=NT)
            for q4 in range(4):
                sl = slice(1152 * q4, 1152 * (q4 + 1))
                if q4 % 2 == 0:
                    nc.scalar.activation(out=y2[:, sl], in_=y2[:, sl],
                                         func=AF.Relu,
                                         bias=ss2[:, 1:2], scale=ss2[:, 0:1])
                else:
                    nc.vector.tensor_scalar(out=y2[:, sl], in0=y2[:, sl],
                                            scalar1=ss2[:, 0:1],
                                            scalar2=ss2[:, 1:2],
                                            op0=OP.mult, op1=OP.add)
                    nc.vector.tensor_scalar(out=y2[:, sl], in0=y2[:, sl],
                                            scalar1=0.0, scalar2=None,
                                            op0=OP.max)
                ylo = y2[0:64, sl].rearrange("c (g t) -> c g t", t=NT)
                yhi = y2[64:128, sl].rearrange("c (g t) -> c g t", t=NT)
                nc.sync.dma_start(out=yv2[:, 3 * q4:3 * q4 + 3, 0, :], in_=ylo)
                nc.sync.dma_start(out=yv2[:, 3 * q4:3 * q4 + 3, 1, :], in_=yhi)
        stk.close()
    nc.compile()
    _CACHE["nc"] = nc
    return nc


# ---------------------------------------------------------------- entry point
def kernel(cen, sum_weights, q_w, k_w, v_w, out_w, bn_gamma, bn_beta):
    from concourse.bass_utils import run_bass_kernel_spmd
    import ml_dtypes
    cen = np.asarray(cen, np.float32)
    wconvk, wconvv, wcvT, wq, outwp, stack2, ind4, ind4T = _host_weights(
        np.asarray(sum_weights), np.asarray(q_w),
        np.asarray(k_w), np.asarray(v_w), np.asarray(out_w))
    idb = np.eye(128, dtype=ml_dtypes.bfloat16)
    idr = np.eye(64, dtype=np.float32)
    idf = np.eye(128, dtype=np.float32)
    gam = np.asarray(bn_gamma, np.float32).reshape(64, 1)
    bet = np.asarray(bn_beta, np.float32).reshape(64, 1)

    import time as _t
    _t0 = _t.time()
    nc = _build_nc()
    print(f"[kernel] build+compile: {_t.time() - _t0:.1f}s", flush=True)
    in_maps = []
    for b in range(B):
        xp = np.zeros((C, PW, PW), np.float32)
        xp[:, PAD:PAD + H, PAD:PAD + W] = cen[b]
        in_maps.append({
            "xpad": xp.reshape(C, PW * PW).astype(ml_dtypes.bfloat16),
            "wconvk": wconvk, "wconvv": wconvv, "wconvvT": wcvT, "wq": wq,
            "outwp": outwp, "stack2": stack2, "ind4": ind4, "ind4T": ind4T,
            "idb": idb, "idr": idr, "idf": idf,
            "gamma": gam, "beta": bet,
        })
    trace = bool(int(os.environ.get("KERNEL_TRACE", "0")))
    res = run_bass_kernel_spmd(nc, in_maps, core_ids=list(range(NCORES)),
                               trace=trace)
    kernel.last_exec_time_ns = res.exec_time_ns
    out = np.stack([res.results[b]["yout"].reshape(64, H, W) for b in range(B)])
    return out.astype(np.float32)
